# revision 4
# baseline (speedup 1.0000x reference)
"""BBoxTargetExpand on 8 TRN2 NeuronCores.

The reference is `where(labels > 0, x, x)` for both float tensors — an
identity copy. So the device kernel is a pure HBM->HBM memcpy of the two
f32 tensors, sharded over rows across the 8 cores; `labels` never needs
to touch the device.

Device kernel: one InstDMACopy per tensor, issued on the two HWDGE rings
(sync/SP for bbox_targets, scalar/ACT for bbox_weights) so descriptor
generation runs in parallel. Each InstDMACopy is sprayed by HWDGE across
all 16 SDMA engines (~21.4 GB/s per engine sustained, which saturates
the per-NC HBM share). no_gpsimd_drain skips the Pool-engine DGE drain
in the block epilogue — no SWDGE DMAs are ever issued.
"""

import sys
import types

import numpy as np

import concourse.bass as bass
import concourse.mybir as mybir
from concourse.bass_utils import run_bass_kernel_spmd


def _ensure_ntff_hook_importable():
    """bass_utils does `from antenv.axon_hooks import get_axon_ntff_profile_hook`
    when tracing is requested (e.g. BASS_TRACE=1 in the environment). Some agent
    images lack that module; install a best-effort shim so tracing either works
    (via the ctypes hook from trn_boot) or degrades gracefully instead of
    crashing with ModuleNotFoundError."""
    try:
        import antenv.axon_hooks  # noqa: F401

        return
    except ImportError:
        pass
    try:
        from trn_agent_boot.trn_boot import _ntff_profile_via_ctypes

        hook = _ntff_profile_via_ctypes("/opt/axon/libaxon_pjrt.so")
    except Exception:
        hook = None
    mod = types.ModuleType("antenv.axon_hooks")
    mod.get_axon_ntff_profile_hook = lambda: hook
    sys.modules["antenv.axon_hooks"] = mod


_ensure_ntff_hook_importable()

M = 8_000_000
N = 4
N_CORES = 8
M_SHARD = M // N_CORES          # 1_000_000 rows per core
ELEMS = M_SHARD * N             # 4_000_000 f32 = 16 MiB per tensor per core

_nc_cache = None


def _build():
    global _nc_cache
    if _nc_cache is not None:
        return _nc_cache
    nc = bass.Bass()
    t_in = nc.declare_dram_parameter("t_in", [ELEMS], mybir.dt.float32, isOutput=False)
    w_in = nc.declare_dram_parameter("w_in", [ELEMS], mybir.dt.float32, isOutput=False)
    t_out = nc.declare_dram_parameter("t_out", [ELEMS], mybir.dt.float32, isOutput=True)
    w_out = nc.declare_dram_parameter("w_out", [ELEMS], mybir.dt.float32, isOutput=True)

    with (
        nc.Block(no_gpsimd_drain=True) as block,
        nc.semaphore("sem_t") as sem_t,
        nc.semaphore("sem_w") as sem_w,
    ):

        @block.sync
        def _(sync: bass.BassEngine):
            sync.dma_start(out=t_out[:], in_=t_in[:]).then_inc(sem_t, 16)
            sync.wait_ge(sem_t, 16)

        @block.scalar
        def _(scalar: bass.BassEngine):
            scalar.dma_start(out=w_out[:], in_=w_in[:]).then_inc(sem_w, 16)
            scalar.wait_ge(sem_w, 16)

    _nc_cache = nc
    return nc


def _run(bbox_targets, bbox_weights, **kwargs):
    nc = _build()
    t = np.ascontiguousarray(np.asarray(bbox_targets, dtype=np.float32)).reshape(
        N_CORES, ELEMS
    )
    w = np.ascontiguousarray(np.asarray(bbox_weights, dtype=np.float32)).reshape(
        N_CORES, ELEMS
    )
    in_maps = [{"t_in": t[c], "w_in": w[c]} for c in range(N_CORES)]
    res = run_bass_kernel_spmd(nc, in_maps, list(range(N_CORES)), **kwargs)
    t_out = np.concatenate(
        [res.results[c]["t_out"] for c in range(N_CORES)]
    ).reshape(M, N)
    w_out = np.concatenate(
        [res.results[c]["w_out"] for c in range(N_CORES)]
    ).reshape(M, N)
    return (t_out, w_out), res


def kernel(bbox_targets, bbox_weights, labels=None, **kwargs):
    (t_out, w_out), _ = _run(bbox_targets, bbox_weights)
    return (t_out, w_out)


# revision 5
# speedup vs baseline: 1.1909x; 1.1909x over previous
"""BBoxTargetExpand on 8 TRN2 NeuronCores.

The reference is `where(labels > 0, x, x)` for both float tensors — an
identity copy. So the device kernel is a pure HBM->HBM memcpy of the two
f32 tensors, sharded over rows across the 8 cores; `labels` never needs
to touch the device.

Device kernel: one InstDMACopy per tensor, issued on the two HWDGE rings
(sync/SP for bbox_targets, scalar/ACT for bbox_weights) so descriptor
generation runs in parallel. Each InstDMACopy is sprayed by HWDGE across
all 16 SDMA engines (~21.4 GB/s per engine sustained, which saturates
the per-NC HBM share). no_gpsimd_drain skips the Pool-engine DGE drain
in the block epilogue — no SWDGE DMAs are ever issued.
"""

import sys
import types

import numpy as np

import concourse.bass as bass
import concourse.mybir as mybir
from concourse.bass_utils import run_bass_kernel_spmd


def _ensure_ntff_hook_importable():
    """bass_utils does `from antenv.axon_hooks import get_axon_ntff_profile_hook`
    when tracing is requested (e.g. BASS_TRACE=1 in the environment). Some agent
    images lack that module; install a best-effort shim so tracing either works
    (via the ctypes hook from trn_boot) or degrades gracefully instead of
    crashing with ModuleNotFoundError."""
    try:
        import antenv.axon_hooks  # noqa: F401

        return
    except ImportError:
        pass
    try:
        from trn_agent_boot.trn_boot import _ntff_profile_via_ctypes

        hook = _ntff_profile_via_ctypes("/opt/axon/libaxon_pjrt.so")
    except Exception:
        hook = None
    mod = types.ModuleType("antenv.axon_hooks")
    mod.get_axon_ntff_profile_hook = lambda: hook
    sys.modules["antenv.axon_hooks"] = mod


_ensure_ntff_hook_importable()

M = 8_000_000
N = 4
N_CORES = 8
M_SHARD = M // N_CORES          # 1_000_000 rows per core
ELEMS = M_SHARD * N             # 4_000_000 f32 = 16 MiB per tensor per core

_nc_cache = None


def _build():
    global _nc_cache
    if _nc_cache is not None:
        return _nc_cache
    # partition_id is never read by this kernel; disabling it drops one input
    # tensor binding per core from every dispatch.
    nc = bass.Bass(enable_partition_id=False)
    t_in = nc.declare_dram_parameter("t_in", [ELEMS], mybir.dt.float32, isOutput=False)
    w_in = nc.declare_dram_parameter("w_in", [ELEMS], mybir.dt.float32, isOutput=False)
    t_out = nc.declare_dram_parameter("t_out", [ELEMS], mybir.dt.float32, isOutput=True)
    w_out = nc.declare_dram_parameter("w_out", [ELEMS], mybir.dt.float32, isOutput=True)

    with (
        nc.Block(no_gpsimd_drain=True) as block,
        nc.semaphore("sem_t") as sem_t,
        nc.semaphore("sem_w") as sem_w,
    ):

        @block.sync
        def _(sync: bass.BassEngine):
            sync.dma_start(out=t_out[:], in_=t_in[:]).then_inc(sem_t, 16)
            sync.wait_ge(sem_t, 16)

        @block.scalar
        def _(scalar: bass.BassEngine):
            scalar.dma_start(out=w_out[:], in_=w_in[:]).then_inc(sem_w, 16)
            scalar.wait_ge(sem_w, 16)

    _nc_cache = nc
    return nc


def _run(bbox_targets, bbox_weights, **kwargs):
    nc = _build()
    t = np.ascontiguousarray(np.asarray(bbox_targets, dtype=np.float32)).reshape(
        N_CORES, ELEMS
    )
    w = np.ascontiguousarray(np.asarray(bbox_weights, dtype=np.float32)).reshape(
        N_CORES, ELEMS
    )
    in_maps = [{"t_in": t[c], "w_in": w[c]} for c in range(N_CORES)]
    res = run_bass_kernel_spmd(nc, in_maps, list(range(N_CORES)), **kwargs)
    t_out = np.concatenate(
        [res.results[c]["t_out"] for c in range(N_CORES)]
    ).reshape(M, N)
    w_out = np.concatenate(
        [res.results[c]["w_out"] for c in range(N_CORES)]
    ).reshape(M, N)
    return (t_out, w_out), res


def kernel(bbox_targets, bbox_weights, labels=None, **kwargs):
    (t_out, w_out), _ = _run(bbox_targets, bbox_weights)
    return (t_out, w_out)
